# revision 1
# baseline (speedup 1.0000x reference)
"""Multi-head self-attention on 8 TRN2 NeuronCores — v3.

Same math/sharding as v1/v2 ((batch x query-half) shards, fp16 compute,
transposed-scores softmax with ones-column denominators), but restructured
for this platform's cost profile (measured: ~50us per matmul instruction,
~10us per DVE op, ACT ~free): matmul count minimized via N=1024 moving
operands, all inputs shipped in exact SBUF layout and loaded with one
contiguous DMA each.
"""

import os
import numpy as np

B, S, D = 4, 2048, 1024
H, DK = 16, 64
SQ = S // 2
FV = H * 65          # V' columns incl. per-head ones column
SCALE = 64 ** -0.5
NCORES = 8

_cache = {}
LAST_EXEC_TIME_NS = None

MMN = int(os.environ.get("KERNEL_MMN", "512"))   # moving free dim per matmul


def _build_nc(repeat=1):
    import concourse.bass as bass
    import concourse.mybir as mybir
    import concourse.tile as tile
    from concourse import bacc

    fp16 = mybir.dt.float16
    f32 = mybir.dt.float32
    mult = mybir.AluOpType.mult
    add = mybir.AluOpType.add

    nc = bacc.Bacc(target_bir_lowering=False, debug=False, num_devices=NCORES)

    # ---- DRAM parameters, already in SBUF layout ----
    xt_d = nc.dram_tensor("xt", [128, 8, S], fp16, kind="ExternalInput")
    xq_d = nc.dram_tensor("xq", [128, 8, SQ], fp16, kind="ExternalInput")
    wq_d = nc.dram_tensor("wq", [128, 64, 128], fp16, kind="ExternalInput")
    wk_d = nc.dram_tensor("wk", [128, 64, 128], fp16, kind="ExternalInput")
    wv_d = nc.dram_tensor("wv", [128, 8, 1024], fp16, kind="ExternalInput")  # dense V wT
    bq_d = nc.dram_tensor("bq", [128, 8], f32, kind="ExternalInput")
    bk_d = nc.dram_tensor("bk", [128, 8], f32, kind="ExternalInput")
    bv_d = nc.dram_tensor("bv", [1024], fp16, kind="ExternalInput")          # dense V bias
    pw_d = nc.dram_tensor("pw", [128, 8, 1024], fp16, kind="ExternalInput")
    pb_d = nc.dram_tensor("pb", [1024], f32, kind="ExternalInput")
    out_d = nc.dram_tensor("out", [SQ, D], f32, kind="ExternalOutput")

    def bcast_rows(ap, parts):
        return bass.AP(tensor=ap.tensor, offset=ap.offset, ap=[[0, parts], *ap.ap])

    def mm_chunks(total):
        c = []
        o = 0
        while o < total:
            n = min(MMN, total - o)
            c.append((o, n))
            o += n
        return c

    with tile.TileContext(nc) as tc:
        with (
            tc.tile_pool(name="const", bufs=1) as const,
            tc.tile_pool(name="xpool", bufs=1) as xpool,
            tc.tile_pool(name="acts", bufs=1) as acts,
            tc.tile_pool(name="qk", bufs=2) as qkpool,
            tc.tile_pool(name="estream", bufs=4) as estream,
            tc.tile_pool(name="small", bufs=3) as small,
            tc.tile_pool(name="ps", bufs=2, space="PSUM") as ps,
            tc.tile_pool(name="psO", bufs=2, space="PSUM") as psO,
            tc.tile_pool(name="dscr", bufs=2, space="DRAM") as dscr,
        ):
            bvb = const.tile([128, 1024], fp16, tag="bvb")
            nc.sync.dma_start(out=bvb, in_=bcast_rows(bv_d.ap(), 128))
            pbb = const.tile([128, 1024], f32, tag="pbb")
            nc.sync.dma_start(out=pbb, in_=bcast_rows(pb_d.ap(), 128))
            wq_all = const.tile([128, 64, 128], fp16, tag="wq_all")
            nc.sync.dma_start(out=wq_all, in_=wq_d.ap())
            wk_all = const.tile([128, 64, 128], fp16, tag="wk_all")
            nc.sync.dma_start(out=wk_all, in_=wk_d.ap())
            bq_all = const.tile([128, 8], f32, tag="bq_all")
            nc.sync.dma_start(out=bq_all, in_=bq_d.ap())
            bk_all = const.tile([128, 8], f32, tag="bk_all")
            nc.sync.dma_start(out=bk_all, in_=bk_d.ap())

            def body():
                xt = xpool.tile([128, 8, S], fp16, tag="xt", name="xt")
                nc.sync.dma_start(out=xt, in_=xt_d.ap())
                xq = xpool.tile([128, 8, SQ], fp16, tag="xq", name="xq")
                nc.sync.dma_start(out=xq, in_=xq_d.ap())
                # pw later reuses wv's slot (same tag) once V' is done
                wv = xpool.tile([128, 8, 1024], fp16, tag="wv", name="wv", bufs=1)
                nc.sync.dma_start(out=wv, in_=wv_d.ap())

                # ---- V' ----
                vt = []
                for st in range(16):
                    psa = ps.tile([128, 1024], f32, tag="ps", name="psa")
                    for dt in range(8):
                        for o, n in mm_chunks(1024):
                            nc.tensor.matmul(psa[:, o:o + n],
                                             xt[:, dt, st * 128:(st + 1) * 128],
                                             wv[:, dt, o:o + n],
                                             start=(dt == 0), stop=(dt == 7))
                    v = acts.tile([128, 16, 65], fp16, tag=f"v{st}", name=f"v{st}")
                    # dense [128,1024] psum + bias -> strided 64-col blocks of v
                    nc.vector.tensor_tensor(
                        v[:, :, 0:64],
                        psa.rearrange("p (a b) -> p a b", a=16),
                        bvb.rearrange("p (a b) -> p a b", a=16), add)
                    nc.vector.memset(v[:, :, 64], 1.0)
                    vt.append(v)

                otn = [acts.tile([128, SQ], fp16, tag=f"otn{i}", name=f"otn{i}")
                       for i in range(8)]

                def qk_pair(hp):
                    psq = ps.tile([128, SQ], f32, tag="ps", name="psq")
                    for dt in range(8):
                        for o, n in mm_chunks(SQ):
                            nc.tensor.matmul(psq[:, o:o + n],
                                             wq_all[:, hp * 8 + dt, :],
                                             xq[:, dt, o:o + n],
                                             start=(dt == 0), stop=(dt == 7))
                    qt = qkpool.tile([128, SQ], fp16, tag="qt", name="qt")
                    nc.vector.tensor_scalar(qt[:], psq, bq_all[:, hp:hp + 1], None, add)

                    kt_t = qkpool.tile([128, S], fp16, tag="kt", name="kt_t")
                    # dt outer: each wk stationary serves both sequence halves
                    # (4 matmuls) before swapping; both psum tiles accumulate
                    # in parallel across the dt loop (2 ps slots)
                    psk2 = [ps.tile([128, SQ], f32, tag="ps", name=f"psk{h}")
                            for h in range(2)]
                    for dt in range(8):
                        for half in range(2):
                            for o, n in mm_chunks(SQ):
                                nc.tensor.matmul(psk2[half][:, o:o + n],
                                                 wk_all[:, hp * 8 + dt, :],
                                                 xt[:, dt, half * SQ + o:half * SQ + o + n],
                                                 start=(dt == 0), stop=(dt == 7))
                    for half in range(2):
                        nc.vector.tensor_scalar(kt_t[:, half * SQ:(half + 1) * SQ],
                                                psk2[half], bk_all[:, hp:hp + 1], None, add)
                    return qt, kt_t

                def attention(hp, qt, kt_t):
                    ot2 = []
                    for hh in range(2):
                        ot = psO.tile([65, SQ], f32, tag="ot", name=f"ot{hh}")
                        ot2.append(ot)
                    for kt in range(16):
                        sc2 = []
                        for hh in range(2):
                            sc = ps.tile([128, SQ], f32, tag="ps", name=f"sc{hh}")
                            sc2.append(sc)
                        # chunks inner per head: consecutive matmuls share the
                        # stationary (KT slice) -> half the stationary swaps
                        for hh in range(2):
                            hsl = slice(hh * 64, (hh + 1) * 64)
                            for o, n in mm_chunks(SQ):
                                nc.tensor.matmul(
                                    sc2[hh][:, o:o + n],
                                    kt_t[hsl, kt * 128:(kt + 1) * 128],
                                    qt[hsl, o:o + n],
                                    start=True, stop=True)
                        for hh in range(2):
                            h = 2 * hp + hh
                            e = estream.tile([128, SQ], fp16, tag="e", name="e")
                            nc.scalar.activation(e[:], sc2[hh][:],
                                                 mybir.ActivationFunctionType.Exp,
                                                 scale=float(SCALE))
                            for o, n in mm_chunks(SQ):
                                nc.tensor.matmul(
                                    ot2[hh][:, o:o + n],
                                    vt[kt][:, h, :],
                                    e[:, o:o + n],
                                    start=(kt == 0), stop=(kt == 15))
                    for hh in range(2):
                        ot = ot2[hh]
                        rec = small.tile([1, SQ], f32, tag="rec", name="rec")
                        nc.vector.reciprocal(rec, ot[64:65, :])
                        recb = small.tile([64, SQ], f32, tag="recb", name="recb")
                        nc.gpsimd.partition_broadcast(recb, rec)
                        nc.vector.tensor_tensor(otn[hp][hh * 64:(hh + 1) * 64, :],
                                                ot[0:64, :], recb, mult)

                pend = qk_pair(0)
                for hp in range(8):
                    nxt = qk_pair(hp + 1) if hp < 7 else None
                    attention(hp, *pend)
                    pend = nxt

                # ---- output projection ----
                pw = xpool.tile([128, 8, 1024], fp16, tag="wv", name="pw", bufs=1)
                nc.sync.dma_start(out=pw, in_=pw_d.ap())
                for st in range(8):
                    pso = ps.tile([128, 1024], f32, tag="ps", name="pso")
                    for ft in range(8):
                        for o, n in mm_chunks(1024):
                            nc.tensor.matmul(pso[:, o:o + n],
                                             otn[ft][:, st * 128:(st + 1) * 128],
                                             pw[:, ft, o:o + n],
                                             start=(ft == 0), stop=(ft == 7))
                    o_t = small.tile([128, 1024], f32, tag="o_t", name="o_t", bufs=2)
                    nc.vector.tensor_tensor(o_t, pso, pbb, add)
                    nc.sync.dma_start(out=out_d.ap()[st * 128:(st + 1) * 128, :], in_=o_t)

            for _rep in range(repeat):
                body()

    nc.compile()
    return nc


def _prep_shared(qkv_w, qkv_b, proj_w, proj_b):
    f16 = np.float16
    wqT = np.ascontiguousarray(qkv_w[0:1024].T)          # [D, 1024]
    wkT = np.ascontiguousarray(qkv_w[1024:2048].T)
    wvT = np.ascontiguousarray(qkv_w[2048:3072].T)
    # wq_all[p, hp*8+dt, c] = wqT[dt*128+p, hp*128+c]
    wq = np.ascontiguousarray(
        wqT.reshape(8, 128, 8, 128).transpose(1, 2, 0, 3).reshape(128, 64, 128)).astype(f16)
    wk = np.ascontiguousarray(
        wkT.reshape(8, 128, 8, 128).transpose(1, 2, 0, 3).reshape(128, 64, 128)).astype(f16)
    # wv[p, dt, f] = wvT[dt*128+p, f] ; V' ones handled on-device by memset
    wv = np.ascontiguousarray(
        wvT.reshape(8, 128, 1024).transpose(1, 0, 2)).astype(f16)
    pw = np.ascontiguousarray(
        proj_w.T.reshape(8, 128, 1024).transpose(1, 0, 2)).astype(f16)
    bq = np.ascontiguousarray(qkv_b[0:1024].reshape(8, 128).T).astype(np.float32)
    bk = np.ascontiguousarray(qkv_b[1024:2048].reshape(8, 128).T).astype(np.float32)
    return dict(
        wq=wq, wk=wk, wv=wv, bq=bq, bk=bk,
        bv=np.ascontiguousarray(qkv_b[2048:3072]).astype(f16),
        pw=pw,
        pb=np.ascontiguousarray(proj_b).astype(np.float32),
    )


def _make_in_maps(x, qkv_w, qkv_b, proj_w, proj_b):
    x = np.asarray(x, np.float32)
    shared = _prep_shared(np.asarray(qkv_w, np.float32), np.asarray(qkv_b, np.float32),
                          np.asarray(proj_w, np.float32), np.asarray(proj_b, np.float32))
    in_maps = []
    for c in range(NCORES):
        b, half = c // 2, c % 2
        xT = np.ascontiguousarray(x[b].T).astype(np.float16)          # [D, S]
        m = dict(shared)
        m["xt"] = np.ascontiguousarray(xT.reshape(8, 128, S).transpose(1, 0, 2))
        m["xq"] = np.ascontiguousarray(
            xT[:, half * SQ:(half + 1) * SQ].reshape(8, 128, SQ).transpose(1, 0, 2))
        in_maps.append(m)
    return in_maps


def kernel(x, qkv_w, qkv_b, proj_w, proj_b):
    global LAST_EXEC_TIME_NS
    from concourse.bass_utils import run_bass_kernel_spmd

    in_maps = _make_in_maps(x, qkv_w, qkv_b, proj_w, proj_b)
    if "nc" not in _cache:
        _cache["nc"] = _build_nc()
    nc = _cache["nc"]

    res = run_bass_kernel_spmd(nc, in_maps, core_ids=list(range(NCORES)))
    LAST_EXEC_TIME_NS = res.exec_time_ns

    out = np.zeros((B, S, D), np.float32)
    for c in range(NCORES):
        b, half = c // 2, c % 2
        out[b, half * SQ:(half + 1) * SQ, :] = res.results[c]["out"]
    return out



# revision 14
# speedup vs baseline: 1.3067x; 1.3067x over previous
"""Multi-head self-attention on 8 TRN2 NeuronCores — v4 (hardware loops).

This stack's cost profile (measured): ~110us per STATIC instruction,
~20-25us per For_i back-edge, dynamic instruction execution ~free.
So the kernel is restructured as For_i hardware loops with small static
bodies (~170 static instructions vs ~3700 unrolled in v3).

Sharding: core = (batch b, head-group hg of 8 heads). Each core computes
Q/K/V + attention for its 8 heads over the full 2048-seq batch, plus the
partial output projection over its 512 fv columns. Host sums the two
partials per batch and adds proj bias.

Stationary operands can't be register-indexed (ldweights limitation), so
loop bodies copy the stationary chunk into a fixed scratch tile (DVE) and
matmul from there. PSUM accumulation across loop iterations uses
memset + start=False (has_written semantics make this correct whether or
not the bank was previously written).
"""

import numpy as np

B, S, D = 4, 2048, 1024
H, DK = 16, 64
SCALE = 64 ** -0.5
NCORES = 8

_cache = {}
LAST_EXEC_TIME_NS = None


def _build_nc(repeat=1, debug_taps=False):
    import concourse.bass as bass
    import concourse.mybir as mybir
    import concourse.tile as tile
    from concourse import bacc

    ds = bass.ds
    fp16 = mybir.dt.float16
    f32 = mybir.dt.float32
    mult = mybir.AluOpType.mult
    add = mybir.AluOpType.add

    nc = bacc.Bacc(target_bir_lowering=False, debug=False, num_devices=NCORES)

    xt_d = nc.dram_tensor("xt", [128, 8, S], fp16, kind="ExternalInput")
    wqk_d = nc.dram_tensor("wqk", [128, 8, 1024], fp16, kind="ExternalInput")
    wv_d = nc.dram_tensor("wv", [128, 8, 512], fp16, kind="ExternalInput")
    bqk_d = nc.dram_tensor("bqk", [128, 8], f32, kind="ExternalInput")
    bv_d = nc.dram_tensor("bv", [512], fp16, kind="ExternalInput")
    pw_d = nc.dram_tensor("pw", [128, 4, 1024], fp16, kind="ExternalInput")
    out_d = nc.dram_tensor("out", [128, 8, S], f32, kind="ExternalOutput")
    if debug_taps:
        dbg_qk_d = nc.dram_tensor("dbg_qk", [128, 8, S], fp16,
                                  kind="ExternalOutput")
        dbg_vt_d = nc.dram_tensor("dbg_vt", [128, 16, 8, 65], fp16,
                                  kind="ExternalOutput")
        dbg_otn_d = nc.dram_tensor("dbg_otn", [128, 4, S], fp16,
                                   kind="ExternalOutput")
        dbg_ot_d = nc.dram_tensor("dbg_ot", [128, 2, 512], f32,
                                  kind="ExternalOutput")
        dbg_e_d = nc.dram_tensor("dbg_e", [128, 2, 512], fp16,
                                 kind="ExternalOutput")
        dbg_ks_d = nc.dram_tensor("dbg_ks", [128, 2, 128], fp16,
                                  kind="ExternalOutput")
        dbg_vs_d = nc.dram_tensor("dbg_vs", [128, 2, 65], fp16,
                                  kind="ExternalOutput")

    def bcast_rows(ap, parts):
        return bass.AP(tensor=ap.tensor, offset=ap.offset, ap=[[0, parts], *ap.ap])

    with tile.TileContext(nc) as tc:
        with (
            tc.tile_pool(name="sb", bufs=1) as sb,
            tc.tile_pool(name="ps", bufs=1, space="PSUM") as ps,
        ):
            xt = sb.tile([128, 8, S], fp16, tag="xt")
            nc.sync.dma_start(out=xt, in_=xt_d.ap())
            wqk = sb.tile([128, 8, 1024], fp16, tag="wqk")
            nc.sync.dma_start(out=wqk, in_=wqk_d.ap())
            wv = sb.tile([128, 8, 512], fp16, tag="wv")
            nc.sync.dma_start(out=wv, in_=wv_d.ap())
            bqk = sb.tile([128, 8], f32, tag="bqk")
            nc.sync.dma_start(out=bqk, in_=bqk_d.ap())
            bvb = sb.tile([128, 512], fp16, tag="bvb")
            nc.sync.dma_start(out=bvb, in_=bcast_rows(bv_d.ap(), 128))
            pw = sb.tile([128, 4, 1024], fp16, tag="pw")
            nc.sync.dma_start(out=pw, in_=pw_d.ap())

            qkT = sb.tile([128, 8, S], fp16, tag="qkT")     # fc 0-3 Q, 4-7 K
            vt = sb.tile([128, 16, 8, 65], fp16, tag="vt")  # [p, st, hh, dv+1]
            otn = sb.tile([128, 4, S], fp16, tag="otn")     # [dv|hp, q]
            outS = sb.tile([128, 8, 512], f32, tag="outS")

            ws = sb.tile([128, 1024], fp16, tag="ws")
            xs = sb.tile([128, 128], fp16, tag="xs")
            # all attention scratch tiles are (u, j)-indexed: nothing is
            # written twice within one loop body (avoids within-body WAR)
            ksz, vs, e = {}, {}, {}
            for u in range(2):
                for j in range(2):
                    ksz[u, j] = sb.tile([128, 128], fp16, tag=f"ks{u}{j}",
                                        name=f"ks{u}{j}")
                    vs[u, j] = sb.tile([128, 65], fp16, tag=f"vs{u}{j}",
                                       name=f"vs{u}{j}")
                    e[u, j] = sb.tile([128, 512], fp16, tag=f"e{u}{j}",
                                      name=f"e{u}{j}")
                    # rows of the *other* head stay zero forever: moving
                    # operand then carries both heads' Q; zeros annihilate
                    # the other head's contribution.
                    lo, hi = (64, 128) if j == 0 else (0, 64)
                    nc.vector.memset(ksz[u, j][lo:hi, :], 0.0)
            rec = [sb.tile([1, 512], f32, tag=f"rec{j}", name=f"rec{j}")
                   for j in range(2)]
            recb = [sb.tile([64, 512], f32, tag=f"recb{j}", name=f"recb{j}")
                    for j in range(2)]
            # NOTE: an AP with BOTH a partition offset and a ds() free-dim
            # slice silently drops the partition offset on this stack.
            # ktmp stages the K chunk (full partitions + ds = safe); ksz gets
            # static-AP copies. otnB holds odd heads at base partition 0 and
            # is DMA-shifted into otn rows 64-127 once per body (static APs).
            ktmp = [sb.tile([128, 128], fp16, tag=f"ktmp{u}", name=f"ktmp{u}")
                    for u in range(2)]
            otnB = sb.tile([64, 4, S], fp16, tag="otnB")
            otnBf = otnB.rearrange("p a b -> p (a b)")
            pws = sb.tile([128, 1024], fp16, tag="pws")

            psb = [ps.tile([128, 512], f32, tag=f"p{k}", name=f"psb{k}")
                   for k in range(8)]

            xtf = xt.rearrange("p a b -> p (a b)")
            wqkf = wqk.rearrange("p a b -> p (a b)")
            wvf = wv.rearrange("p a b -> p (a b)")
            qkTf = qkT.rearrange("p a b -> p (a b)")
            vtf = vt.rearrange("p a b c -> p (a b c)")
            otnf = otn.rearrange("p a b -> p (a b)")
            pwf = pw.rearrange("p a b -> p (a b)")

            def body():
                # ---- G1: Q and K, [dk, seq] layouts ----
                with tc.For_i(0, 4, 1, name="g1s") as s:
                    for fc in range(8):
                        nc.vector.memset(psb[fc], 0.0)
                    with tc.For_i(0, 8, 1, name="g1d") as d:
                        nc.vector.tensor_copy(ws, wqkf[:, ds(d * 1024, 1024)])
                        for fc in range(8):
                            nc.tensor.matmul(
                                psb[fc], ws[:, fc * 128:(fc + 1) * 128],
                                xtf[:, ds(d * S + s * 512, 512)],
                                start=False, stop=False, skip_group_check=True)
                    for fc in range(8):
                        nc.vector.tensor_scalar(
                            qkTf[:, ds(fc * S + s * 512, 512)],
                            psb[fc], bqk[:, fc:fc + 1], None, add)

                # ---- G2: V' in [seq, feat] layout with ones column ----
                with tc.For_i(0, 16, 1, name="g2st") as st:
                    nc.vector.memset(psb[0], 0.0)
                    for dd in range(8):
                        nc.vector.tensor_copy(
                            xs, xtf[:, ds(dd * S + st * 128, 128)])
                        nc.tensor.matmul(
                            psb[0], xs, wvf[:, ds(dd * 512, 512)],
                            start=False, stop=False, skip_group_check=True)
                    nc.vector.tensor_tensor(
                        vt[:, ds(st, 1), :, 0:64],
                        psb[0].rearrange("p (x a b) -> p x a b", x=1, a=8),
                        bvb.rearrange("p (x a b) -> p x a b", x=1, a=8), add)
                    nc.vector.memset(vt[:, ds(st, 1), :, 64:65], 1.0)

                # ---- A: attention (scores -> exp -> weighted V + denom) ----
                with tc.For_i(0, 4, 1, name="aqc") as qc:
                    with tc.For_i(0, 4, 1, name="ahp") as hp:
                        nc.vector.memset(psb[2], 0.0)
                        nc.vector.memset(psb[3], 0.0)
                        with tc.For_i(0, 8, 1, name="akt") as kti:
                            for u in range(2):
                                nc.vector.tensor_copy(
                                    ktmp[u], qkTf[:, ds(hp * S + kti * 256
                                                        + 8192 + u * 128, 128)])
                                for j in range(2):
                                    lo, hi = j * 64, (j + 1) * 64
                                    nc.vector.tensor_copy(
                                        ksz[u, j][lo:hi, :], ktmp[u][lo:hi, :])
                                for j in range(2):
                                    sc = psb[j + 4 * u]  # u0: p0/p1, u1: p4/p5
                                    nc.tensor.matmul(
                                        sc, ksz[u, j],
                                        qkTf[:, ds(hp * S + qc * 512, 512)],
                                        start=True, stop=True)
                                    nc.scalar.activation(
                                        e[u, j], sc,
                                        mybir.ActivationFunctionType.Exp,
                                        scale=float(SCALE))
                                    nc.vector.tensor_copy(
                                        vs[u, j],
                                        vtf[:, ds(kti * 1040 + hp * 130
                                                  + u * 520 + j * 65, 65)])
                                    nc.tensor.matmul(
                                        psb[2 + j][0:65, :], vs[u, j], e[u, j],
                                        start=False, stop=False,
                                        skip_group_check=True)
                        for j in range(2):
                            nc.vector.reciprocal(rec[j], psb[2 + j][64:65, :])
                            nc.gpsimd.partition_broadcast(recb[j], rec[j])
                            dst = otnf if j == 0 else otnBf
                            nc.vector.tensor_tensor(
                                dst[0:64, ds(hp * S + qc * 512, 512)],
                                psb[2 + j][0:64, :], recb[j], mult)
                for fc in range(4):
                    nc.sync.dma_start(out=otn[64:128, fc, :],
                                      in_=otnB[:, fc, :])

                # ---- P: partial output projection [d, seq] ----
                with tc.For_i(0, 4, 1, name="ps_") as s:
                    for dt in range(8):
                        nc.vector.memset(psb[dt], 0.0)
                    with tc.For_i(0, 4, 1, name="pfc") as fc:
                        nc.vector.tensor_copy(pws, pwf[:, ds(fc * 1024, 1024)])
                        for dt in range(8):
                            nc.tensor.matmul(
                                psb[dt], pws[:, dt * 128:(dt + 1) * 128],
                                otnf[:, ds(fc * S + s * 512, 512)],
                                start=False, stop=False, skip_group_check=True)
                    for dt in range(8):
                        nc.vector.tensor_copy(outS[:, dt, :], psb[dt])
                    nc.sync.dma_start(out=out_d.ap()[:, :, ds(s * 512, 512)],
                                      in_=outS)

                if debug_taps:
                    nc.sync.dma_start(out=dbg_qk_d.ap(), in_=qkT)
                    nc.sync.dma_start(out=dbg_vt_d.ap(), in_=vt)
                    nc.sync.dma_start(out=dbg_otn_d.ap(), in_=otn)
                    dbg_ot = sb.tile([128, 2, 512], f32, tag="dbg_ot")
                    for j in range(2):
                        nc.vector.tensor_copy(dbg_ot[0:65, j, :],
                                              psb[2 + j][0:65, :])
                    nc.sync.dma_start(out=dbg_ot_d.ap(), in_=dbg_ot)
                    dbg_e = sb.tile([128, 2, 512], fp16, tag="dbg_e")
                    dbg_ks = sb.tile([128, 2, 128], fp16, tag="dbg_ks")
                    dbg_vs = sb.tile([128, 2, 65], fp16, tag="dbg_vs")
                    for j in range(2):
                        nc.vector.tensor_copy(dbg_e[:, j, :], e[1, j])
                        nc.vector.tensor_copy(dbg_ks[:, j, :], ksz[1, j])
                        nc.vector.tensor_copy(dbg_vs[:, j, :], vs[1, j])
                    nc.sync.dma_start(out=dbg_e_d.ap(), in_=dbg_e)
                    nc.sync.dma_start(out=dbg_ks_d.ap(), in_=dbg_ks)
                    nc.sync.dma_start(out=dbg_vs_d.ap(), in_=dbg_vs)

            for _rep in range(repeat):
                body()

    nc.compile()
    return nc


def _make_in_maps(x, qkv_w, qkv_b, proj_w, proj_b):
    f16 = np.float16
    x = np.asarray(x, np.float32)
    qkv_w = np.asarray(qkv_w, np.float32)
    qkv_b = np.asarray(qkv_b, np.float32)
    proj_w = np.asarray(proj_w, np.float32)

    in_maps = []
    for c in range(NCORES):
        b, hg = c // 2, c % 2
        xT = np.ascontiguousarray(x[b].T)  # [D, S]
        m = {}
        m["xt"] = np.ascontiguousarray(
            xT.reshape(8, 128, S).transpose(1, 0, 2)).astype(f16)
        wq = qkv_w[hg * 512:(hg + 1) * 512]              # [512, D]
        wk = qkv_w[1024 + hg * 512:1024 + (hg + 1) * 512]
        wqk = np.concatenate([wq, wk], axis=0).T          # [D, 1024]
        m["wqk"] = np.ascontiguousarray(
            wqk.reshape(8, 128, 1024).transpose(1, 0, 2)).astype(f16)
        wvv = qkv_w[2048 + hg * 512:2048 + (hg + 1) * 512].T  # [D, 512]
        m["wv"] = np.ascontiguousarray(
            wvv.reshape(8, 128, 512).transpose(1, 0, 2)).astype(f16)
        bq = qkv_b[hg * 512:(hg + 1) * 512].reshape(4, 128).T    # [128, 4]
        bk = qkv_b[1024 + hg * 512:1024 + (hg + 1) * 512].reshape(4, 128).T
        m["bqk"] = np.ascontiguousarray(
            np.concatenate([bq, bk], axis=1)).astype(np.float32)
        m["bv"] = np.ascontiguousarray(
            qkv_b[2048 + hg * 512:2048 + (hg + 1) * 512]).astype(f16)
        pwc = proj_w[:, hg * 512:(hg + 1) * 512].T        # [512 f, 1024 d]
        m["pw"] = np.ascontiguousarray(
            pwc.reshape(4, 128, 1024).transpose(1, 0, 2)).astype(f16)
        in_maps.append(m)
    return in_maps


def kernel(x, qkv_w, qkv_b, proj_w, proj_b):
    global LAST_EXEC_TIME_NS
    from concourse.bass_utils import run_bass_kernel_spmd

    in_maps = _make_in_maps(x, qkv_w, qkv_b, proj_w, proj_b)
    if "nc" not in _cache:
        _cache["nc"] = _build_nc()
    nc = _cache["nc"]

    res = run_bass_kernel_spmd(nc, in_maps, core_ids=list(range(NCORES)))
    LAST_EXEC_TIME_NS = res.exec_time_ns

    pb = np.asarray(proj_b, np.float32)
    out = np.zeros((B, S, D), np.float32)
    for b in range(B):
        acc = None
        for hg in range(2):
            part = res.results[2 * b + hg]["out"]  # [128, 8, S]
            full = part.transpose(1, 0, 2).reshape(D, S)
            acc = full if acc is None else acc + full
        out[b] = acc.T + pb
    return out


# revision 17
# speedup vs baseline: 177.7109x; 136.0030x over previous
"""Multi-head self-attention on 8 TRN2 NeuronCores — v4 (hardware loops).

This stack's cost profile (measured): ~110us per STATIC instruction,
~20-25us per For_i back-edge, dynamic instruction execution ~free.
So the kernel is restructured as For_i hardware loops with small static
bodies (~170 static instructions vs ~3700 unrolled in v3).

Sharding: core = (batch b, head-group hg of 8 heads). Each core computes
Q/K/V + attention for its 8 heads over the full 2048-seq batch, plus the
partial output projection over its 512 fv columns. Host sums the two
partials per batch and adds proj bias.

Stationary operands can't be register-indexed (ldweights limitation), so
loop bodies copy the stationary chunk into a fixed scratch tile (DVE) and
matmul from there. PSUM accumulation across loop iterations uses
memset + start=False (has_written semantics make this correct whether or
not the bank was previously written).
"""

import numpy as np

B, S, D = 4, 2048, 1024
H, DK = 16, 64
SCALE = 64 ** -0.5
NCORES = 8

_cache = {}
LAST_EXEC_TIME_NS = None


def _build_nc(repeat=1, debug_taps=False):
    import concourse.bass as bass
    import concourse.mybir as mybir
    import concourse.tile as tile
    from concourse import bacc

    ds = bass.ds
    fp16 = mybir.dt.float16
    f32 = mybir.dt.float32
    mult = mybir.AluOpType.mult
    add = mybir.AluOpType.add

    nc = bacc.Bacc(target_bir_lowering=False, debug=False, num_devices=NCORES)

    xt_d = nc.dram_tensor("xt", [128, 8, S], fp16, kind="ExternalInput")
    wqk_d = nc.dram_tensor("wqk", [128, 8, 1024], fp16, kind="ExternalInput")
    wv_d = nc.dram_tensor("wv", [128, 8, 512], fp16, kind="ExternalInput")
    bqk_d = nc.dram_tensor("bqk", [128, 8], f32, kind="ExternalInput")
    bv_d = nc.dram_tensor("bv", [512], fp16, kind="ExternalInput")
    pw_d = nc.dram_tensor("pw", [128, 4, 1024], fp16, kind="ExternalInput")
    out_d = nc.dram_tensor("out", [128, 8, S], f32, kind="ExternalOutput")
    if debug_taps:
        dbg_qk_d = nc.dram_tensor("dbg_qk", [128, 8, S], fp16,
                                  kind="ExternalOutput")
        dbg_vt_d = nc.dram_tensor("dbg_vt", [128, 16, 8, 65], fp16,
                                  kind="ExternalOutput")
        dbg_otn_d = nc.dram_tensor("dbg_otn", [128, 4, S], fp16,
                                   kind="ExternalOutput")
        dbg_ot_d = nc.dram_tensor("dbg_ot", [128, 2, 512], f32,
                                  kind="ExternalOutput")
        dbg_e_d = nc.dram_tensor("dbg_e", [128, 2, 512], fp16,
                                 kind="ExternalOutput")
        dbg_ks_d = nc.dram_tensor("dbg_ks", [128, 2, 128], fp16,
                                  kind="ExternalOutput")
        dbg_vs_d = nc.dram_tensor("dbg_vs", [128, 2, 65], fp16,
                                  kind="ExternalOutput")

    def bcast_rows(ap, parts):
        return bass.AP(tensor=ap.tensor, offset=ap.offset, ap=[[0, parts], *ap.ap])

    with tile.TileContext(nc) as tc:
        with (
            tc.tile_pool(name="sb", bufs=1) as sb,
            tc.tile_pool(name="ps", bufs=1, space="PSUM") as ps,
        ):
            xt = sb.tile([128, 8, S], fp16, tag="xt")
            nc.sync.dma_start(out=xt, in_=xt_d.ap())
            wqk = sb.tile([128, 8, 1024], fp16, tag="wqk")
            nc.sync.dma_start(out=wqk, in_=wqk_d.ap())
            wv = sb.tile([128, 8, 512], fp16, tag="wv")
            nc.sync.dma_start(out=wv, in_=wv_d.ap())
            bqk = sb.tile([128, 8], f32, tag="bqk")
            nc.sync.dma_start(out=bqk, in_=bqk_d.ap())
            bvb = sb.tile([128, 512], fp16, tag="bvb")
            nc.sync.dma_start(out=bvb, in_=bcast_rows(bv_d.ap(), 128))
            pw = sb.tile([128, 4, 1024], fp16, tag="pw")
            nc.sync.dma_start(out=pw, in_=pw_d.ap())

            qkT = sb.tile([128, 8, S], fp16, tag="qkT")     # fc 0-3 Q, 4-7 K
            vt = sb.tile([128, 16, 8, 65], fp16, tag="vt")  # [p, st, hh, dv+1]
            otn = sb.tile([128, 4, S], fp16, tag="otn")     # [dv|hp, q]
            outS = sb.tile([128, 8, 512], f32, tag="outS")

            ws = sb.tile([128, 1024], fp16, tag="ws")
            xs = sb.tile([128, 128], fp16, tag="xs")
            # all attention scratch tiles are (u, j)-indexed: nothing is
            # written twice within one loop body (avoids within-body WAR)
            ksz, e = {}, {}
            vsb = [sb.tile([128, 130], fp16, tag=f"vsb{u}", name=f"vsb{u}")
                   for u in range(2)]
            for u in range(2):
                for j in range(2):
                    ksz[u, j] = sb.tile([128, 128], fp16, tag=f"ks{u}{j}",
                                        name=f"ks{u}{j}")
                    e[u, j] = sb.tile([128, 512], fp16, tag=f"e{u}{j}",
                                      name=f"e{u}{j}")
                    # rows of the *other* head stay zero forever: moving
                    # operand then carries both heads' Q; zeros annihilate
                    # the other head's contribution.
                    lo, hi = (64, 128) if j == 0 else (0, 64)
                    nc.vector.memset(ksz[u, j][lo:hi, :], 0.0)
            rec = [sb.tile([1, 512], f32, tag=f"rec{j}", name=f"rec{j}")
                   for j in range(2)]
            recb = [sb.tile([64, 512], f32, tag=f"recb{j}", name=f"recb{j}")
                    for j in range(2)]
            # NOTE: an AP with BOTH a partition offset and a ds() free-dim
            # slice silently drops the partition offset on this stack.
            # ktmp stages the K chunk (full partitions + ds = safe); ksz gets
            # static-AP copies. otnB holds odd heads at base partition 0 and
            # is DMA-shifted into otn rows 64-127 once per body (static APs).
            ktmp = [sb.tile([128, 128], fp16, tag=f"ktmp{u}", name=f"ktmp{u}")
                    for u in range(2)]
            otnB = sb.tile([64, 4, S], fp16, tag="otnB")
            otnBf = otnB.rearrange("p a b -> p (a b)")
            pws = sb.tile([128, 1024], fp16, tag="pws")

            psb = [ps.tile([128, 512], f32, tag=f"p{k}", name=f"psb{k}")
                   for k in range(8)]

            xtf = xt.rearrange("p a b -> p (a b)")
            wqkf = wqk.rearrange("p a b -> p (a b)")
            wvf = wv.rearrange("p a b -> p (a b)")
            qkTf = qkT.rearrange("p a b -> p (a b)")
            vtf = vt.rearrange("p a b c -> p (a b c)")
            otnf = otn.rearrange("p a b -> p (a b)")
            pwf = pw.rearrange("p a b -> p (a b)")

            Copy = mybir.ActivationFunctionType.Copy

            def body():
                # ---- G1: Q and K, [dk, seq] layouts (flattened s x d) ----
                with tc.For_i(0, 32, 1, name="g1") as i1:
                    d = i1 % 8
                    s = i1 // 8
                    with tc.If(d == 0, name="g1pre"):
                        for fc in range(8):
                            nc.vector.memset(psb[fc], 0.0)
                    nc.gpsimd.tensor_copy(ws, wqkf[:, ds(d * 1024, 1024)])
                    for fc in range(8):
                        nc.tensor.matmul(
                            psb[fc], ws[:, fc * 128:(fc + 1) * 128],
                            xtf[:, ds(d * S + s * 512, 512)],
                            start=False, stop=False, skip_group_check=True)
                    with tc.If(d == 7, name="g1post"):
                        for fc in range(8):
                            nc.vector.tensor_scalar(
                                qkTf[:, ds(fc * S + s * 512, 512)],
                                psb[fc], bqk[:, fc:fc + 1], None, add)

                # ---- G2: V' in [seq, feat] layout with ones column ----
                with tc.For_i(0, 16, 1, name="g2st") as st:
                    nc.vector.memset(psb[0], 0.0)
                    for dd in range(8):
                        nc.scalar.activation(
                            xs, xtf[:, ds(dd * S + st * 128, 128)], Copy)
                        nc.tensor.matmul(
                            psb[0], xs, wvf[:, ds(dd * 512, 512)],
                            start=False, stop=False, skip_group_check=True)
                    nc.vector.tensor_tensor(
                        vt[:, ds(st, 1), :, 0:64],
                        psb[0].rearrange("p (x a b) -> p x a b", x=1, a=8),
                        bvb.rearrange("p (x a b) -> p x a b", x=1, a=8), add)
                    nc.vector.memset(vt[:, ds(st, 1), :, 64:65], 1.0)

                # ---- A: attention, flattened qc x hp x kti ----
                with tc.For_i(0, 128, 1, name="att") as ia:
                    kti = ia % 8
                    hp = (ia // 8) % 4
                    qc = ia // 32
                    with tc.If(kti == 0, name="apre"):
                        nc.vector.memset(psb[2], 0.0)
                        nc.vector.memset(psb[3], 0.0)
                    for u in range(2):
                        nc.gpsimd.tensor_copy(
                            ktmp[u], qkTf[:, ds(hp * S + kti * 256
                                                + 8192 + u * 128, 128)])
                        for j in range(2):
                            lo, hi = j * 64, (j + 1) * 64
                            nc.vector.tensor_copy(
                                ksz[u, j][lo:hi, :], ktmp[u][lo:hi, :])
                        nc.gpsimd.tensor_copy(
                            vsb[u], vtf[:, ds(kti * 1040 + hp * 130
                                              + u * 520, 130)])
                        for j in range(2):
                            sc = psb[j + 4 * u]  # u0: p0/p1, u1: p4/p5
                            nc.tensor.matmul(
                                sc, ksz[u, j],
                                qkTf[:, ds(hp * S + qc * 512, 512)],
                                start=True, stop=True)
                            nc.scalar.activation(
                                e[u, j], sc,
                                mybir.ActivationFunctionType.Exp,
                                scale=float(SCALE))
                            nc.tensor.matmul(
                                psb[2 + j][0:65, :],
                                vsb[u][:, j * 65:(j + 1) * 65], e[u, j],
                                start=False, stop=False,
                                skip_group_check=True)
                    with tc.If(kti == 7, name="apost"):
                        for j in range(2):
                            nc.vector.reciprocal(rec[j], psb[2 + j][64:65, :])
                            nc.gpsimd.partition_broadcast(recb[j], rec[j])
                            dst = otnf if j == 0 else otnBf
                            nc.vector.tensor_tensor(
                                dst[0:64, ds(hp * S + qc * 512, 512)],
                                psb[2 + j][0:64, :], recb[j], mult)
                for fc in range(4):
                    nc.sync.dma_start(out=otn[64:128, fc, :],
                                      in_=otnB[:, fc, :])

                # ---- P: partial output projection, flattened s x fc ----
                with tc.For_i(0, 16, 1, name="proj") as ip:
                    fc = ip % 4
                    s = ip // 4
                    with tc.If(fc == 0, name="ppre"):
                        for dt in range(8):
                            nc.vector.memset(psb[dt], 0.0)
                    nc.gpsimd.tensor_copy(pws, pwf[:, ds(fc * 1024, 1024)])
                    for dt in range(8):
                        nc.tensor.matmul(
                            psb[dt], pws[:, dt * 128:(dt + 1) * 128],
                            otnf[:, ds(fc * S + s * 512, 512)],
                            start=False, stop=False, skip_group_check=True)
                    with tc.If(fc == 3, name="ppost"):
                        for dt in range(8):
                            nc.scalar.activation(outS[:, dt, :], psb[dt], Copy)
                        nc.sync.dma_start(
                            out=out_d.ap()[:, :, ds(s * 512, 512)], in_=outS)

                if debug_taps:
                    nc.sync.dma_start(out=dbg_qk_d.ap(), in_=qkT)
                    nc.sync.dma_start(out=dbg_vt_d.ap(), in_=vt)
                    nc.sync.dma_start(out=dbg_otn_d.ap(), in_=otn)
                    dbg_ot = sb.tile([128, 2, 512], f32, tag="dbg_ot")
                    for j in range(2):
                        nc.vector.tensor_copy(dbg_ot[0:65, j, :],
                                              psb[2 + j][0:65, :])
                    nc.sync.dma_start(out=dbg_ot_d.ap(), in_=dbg_ot)
                    dbg_e = sb.tile([128, 2, 512], fp16, tag="dbg_e")
                    dbg_ks = sb.tile([128, 2, 128], fp16, tag="dbg_ks")
                    dbg_vs = sb.tile([128, 2, 65], fp16, tag="dbg_vs")
                    for j in range(2):
                        nc.vector.tensor_copy(dbg_e[:, j, :], e[1, j])
                        nc.vector.tensor_copy(dbg_ks[:, j, :], ksz[1, j])
                        nc.vector.tensor_copy(dbg_vs[:, j, :], vs[1, j])
                    nc.sync.dma_start(out=dbg_e_d.ap(), in_=dbg_e)
                    nc.sync.dma_start(out=dbg_ks_d.ap(), in_=dbg_ks)
                    nc.sync.dma_start(out=dbg_vs_d.ap(), in_=dbg_vs)

            for _rep in range(repeat):
                body()

    nc.compile()
    return nc


def _make_in_maps(x, qkv_w, qkv_b, proj_w, proj_b):
    f16 = np.float16
    x = np.asarray(x, np.float32)
    qkv_w = np.asarray(qkv_w, np.float32)
    qkv_b = np.asarray(qkv_b, np.float32)
    proj_w = np.asarray(proj_w, np.float32)

    in_maps = []
    for c in range(NCORES):
        b, hg = c // 2, c % 2
        xT = np.ascontiguousarray(x[b].T)  # [D, S]
        m = {}
        m["xt"] = np.ascontiguousarray(
            xT.reshape(8, 128, S).transpose(1, 0, 2)).astype(f16)
        wq = qkv_w[hg * 512:(hg + 1) * 512]              # [512, D]
        wk = qkv_w[1024 + hg * 512:1024 + (hg + 1) * 512]
        wqk = np.concatenate([wq, wk], axis=0).T          # [D, 1024]
        m["wqk"] = np.ascontiguousarray(
            wqk.reshape(8, 128, 1024).transpose(1, 0, 2)).astype(f16)
        wvv = qkv_w[2048 + hg * 512:2048 + (hg + 1) * 512].T  # [D, 512]
        m["wv"] = np.ascontiguousarray(
            wvv.reshape(8, 128, 512).transpose(1, 0, 2)).astype(f16)
        bq = qkv_b[hg * 512:(hg + 1) * 512].reshape(4, 128).T    # [128, 4]
        bk = qkv_b[1024 + hg * 512:1024 + (hg + 1) * 512].reshape(4, 128).T
        m["bqk"] = np.ascontiguousarray(
            np.concatenate([bq, bk], axis=1)).astype(np.float32)
        m["bv"] = np.ascontiguousarray(
            qkv_b[2048 + hg * 512:2048 + (hg + 1) * 512]).astype(f16)
        pwc = proj_w[:, hg * 512:(hg + 1) * 512].T        # [512 f, 1024 d]
        m["pw"] = np.ascontiguousarray(
            pwc.reshape(4, 128, 1024).transpose(1, 0, 2)).astype(f16)
        in_maps.append(m)
    return in_maps


def kernel(x, qkv_w, qkv_b, proj_w, proj_b):
    global LAST_EXEC_TIME_NS
    from concourse.bass_utils import run_bass_kernel_spmd

    in_maps = _make_in_maps(x, qkv_w, qkv_b, proj_w, proj_b)
    if "nc" not in _cache:
        _cache["nc"] = _build_nc()
    nc = _cache["nc"]

    res = run_bass_kernel_spmd(nc, in_maps, core_ids=list(range(NCORES)))
    LAST_EXEC_TIME_NS = res.exec_time_ns
    return _postprocess(res, proj_b)


def _postprocess(res, proj_b):
    pb = np.asarray(proj_b, np.float32)
    out = np.zeros((B, S, D), np.float32)
    for b in range(B):
        acc = None
        for hg in range(2):
            part = res.results[2 * b + hg]["out"]  # [128, 8, S]
            full = part.transpose(1, 0, 2).reshape(D, S)
            acc = full if acc is None else acc + full
        out[b] = acc.T + pb
    return out
